# revision 34
# baseline (speedup 1.0000x reference)
"""AttentionBlock kernel for Trainium2, 8 NeuronCores.

Reference computation (B=4, C=256, H=W=64, TEMB=1024):
    t  = temb @ t_w.T + t_b                       # [B, C]
    q  = q_w @ x + (q_b + t)   (1x1 conv)         # [B, C, HW]
    k  = k_w @ x + (k_b + t)
    v  = v_w @ x + v_b
    att = softmax(q.T k / sqrt(C), axis=j)        # [B, HW, HW]
    hh  = att @ v.T                               # [B, C, HW]
    out = x + p_w @ hh + p_b

Sharding: data-parallel over (batch, query-half): core = b*2 + h.
Each core receives x[b] with its OWN query half rotated to the front
(keys may be processed in any order -- softmax is a sum over keys).
The kernel runs entirely in SBUF: the 67M-entry attention matrix is
never materialized to HBM.

Algebraic folds (all exact, done on the host):
  - k bias (k_b + t): a per-channel shift of k adds a per-QUERY
    constant to every score row, which softmax normalizes away. Gone.
  - v bias: softmax weights sum to 1, so it passes straight through
    the attention average; p_w @ v_b joins p_b in the residual.
  - temb projection: a [B,C] host matvec; the device receives the
    finished q-bias vector.
  - p_w folds into the v conv: W = p_w @ v_w (host), so matmul 2
    accumulates the attention-block output directly.
  - residual + normalization move to the HOST: the device returns the
    UNNORMALIZED hh (bf16) plus the denominator column; the host
    divides and adds x + rbias.  This removes the residual input
    (2MB/core), the reciprocal/stt tail, and lets the output ride a
    single 2KB-line DMA per query block.

Device-side layout (evidence-driven from NTFF traces):
  - everything runs in fp8e4 with MatmulPerfMode.DoubleRow.  Conv
    weights are pre-scaled by 64; q/k/w carry a 64x scale that cancels
    in softmax; 1/(64*64*sqrt(C)) rides the exp input scale.
  - the steady-state pacer in the previous rev was the ACT engine
    (64 exps x 1.11us = 71us vs PE 85us busy).  Now the exp work is
    SPLIT: most key-tile pairs use the ACT spline exp; a subset runs
    on the otherwise-idle DVE as a ONE-instruction Schraudolph at the
    fp8-BIT level: uint8(round(A*s + B)) IS the fp8e4m3 bit pattern of
    ~exp(s*ESCALE).  (fp8 quantization of p was already ~3% per
    element; the bit-level trick folds exp+quantize into one op.)
  - mm2 defers MM2_DEPTH pairs behind mm1+exp so the PE never waits on
    an exp or on the previous block's psum-drain casts.
  - startup: garbage matmuls during the input-DMA wait pre-warm the
    HAM clock gate (else the conv phase runs at 1.2 GHz), and a nested
    "boot" psum pool borrows the still-idle hh accumulator banks so
    the early convs rotate over four psum slots instead of two.
  - x8/weights arrive partition-major (host pre-packed) so every DMA
    line is 2KB contiguous; the output is one [128, 4x257] bf16 tile
    per query block (2056B lines).
"""

import numpy as np
import ml_dtypes
from contextlib import ExitStack

import concourse.bass as bass
import concourse.mybir as mybir
import concourse.tile as tile
from concourse import bacc
from concourse.bass_utils import run_bass_kernel_spmd

F32 = mybir.dt.float32
BF16 = mybir.dt.bfloat16
FP8 = mybir.dt.float8e4
U8 = mybir.dt.uint8
AF = mybir.ActivationFunctionType
DR = mybir.MatmulPerfMode.DoubleRow

B, C, H, W, TEMB = 4, 256, 64, 64, 1024
HW = H * W              # 4096
NQ = HW // 2            # 2048 query pixels per core
N_CORES = 8
WS = 64.0               # fp8 weight pre-scale
ESCALE = (float(C) ** -0.5) / (WS * WS)   # rides the exp instruction

N_CH = HW // 512        # 8 x-chunks of 512 columns
N_JT = HW // 128        # 32 key tiles of 128
N_PAIR = N_JT // 2      # 16 key-tile pairs (DoubleRow contracts 2 at once)
N_IB = NQ // 512        # 4 query blocks of 512
N_KT = C // 128         # 2 channel tiles
CV = C + 1              # wT width: 256 out-channels + 64s col
# fp8-bit Schraudolph: uint8(A8*s + B8) is the fp8e4m3 pattern of
# ~exp(s*ESCALE).  8 bits per octave; 55.5 centers the sawtooth.
SCH_A8 = (8.0 / float(np.log(2.0))) * ESCALE
SCH_B8 = 55.5
# key-tile pairs whose exp runs whole-tile on the DVE (blocks >= 1); the
# rest run on the ACT.  Splitting tiles across engines loses to the DVE's
# ~300ns fixed cost per instruction; whole-tile alternation wins.  Block 0
# keeps every exp on the ACT: during the conv phase the DVE is busy with
# the psum->SBUF weight copies.
DVE_PAIRS = (1, 4, 7, 10, 13)
MM2_DEPTH = 4           # mm2 trails mm1+exp by this many pairs
X8_TILE_CHUNKS = (1, 2, 3, 2)   # x-chunk grouping per DMA instruction


def build_nc():
    nc = bacc.Bacc("TRN2", target_bir_lowering=False, debug=False)

    # ---- DRAM I/O (per-core), all partition-major / 2KB-line friendly ----
    x8_d = nc.dram_tensor("x8", [128, N_CH * 1024], FP8, kind="ExternalInput")
    kwT_d = nc.dram_tensor("kwT", [128, 512], FP8, kind="ExternalInput")
    qwT_d = nc.dram_tensor("qwT", [128, 512], FP8, kind="ExternalInput")
    wwT_d = nc.dram_tensor("wwT", [128, 512], FP8, kind="ExternalInput")
    qb_d = nc.dram_tensor("qb", [128, N_KT], F32, kind="ExternalInput")
    out_d = nc.dram_tensor("out", [128, N_IB * 4 * CV], BF16,
                           kind="ExternalOutput")

    with tile.TileContext(nc) as tc, ExitStack() as ctx:
        const = ctx.enter_context(tc.tile_pool(name="const", bufs=1))
        big = ctx.enter_context(tc.tile_pool(name="big", bufs=1))

        # x8 in 4 tiles (first tile = chunk 0 alone so conv ch0 starts the
        # moment its 128KB lands); every DMA line is 1KB+ contiguous
        x8_t = [big.tile([128, nch, N_KT, 512], FP8, tag=f"x8_{i}",
                         name=f"x8_{i}")
                for i, nch in enumerate(X8_TILE_CHUNKS)]
        x8_of = {}   # chunk -> (tile_idx, sub_idx)
        _c = 0
        for i, nch in enumerate(X8_TILE_CHUNKS):
            for j in range(nch):
                x8_of[_c] = (i, j)
                _c += 1

        def xch(ch):
            i, j = x8_of[ch]
            return x8_t[i][:, j]

        # DMA order is the critical path: k-conv ch0 needs only kwT + x8t0,
        # so those two lead their queues.  sync = [x8t0, kwT, x8 rest],
        # gpsimd = [qb, qwT, wwT].
        def loadw(dram, name, eng):
            t = const.tile([128, N_KT, C], FP8, tag=name)
            eng.dma_start(out=t, in_=dram[:, :])
            return t

        # the HAM warm-up operand memset leads the gpsimd queue: the dummy
        # matmuls depend on it and must start the moment the prologue gate
        # opens, not after the weight-DMA issues
        dumf = big.tile([128, 2, 512], FP8, tag="dumf", name="dumf")
        nc.gpsimd.memset(dumf[:, :, :], 0.0)

        nc.sync.dma_start(out=x8_t[0], in_=x8_d[:, 0:1024])
        kwT = loadw(kwT_d, "kwT", nc.sync)
        qb = const.tile([128, N_KT], F32, tag="qb")
        nc.gpsimd.dma_start(out=qb, in_=qb_d[:, :])
        qwT = loadw(qwT_d, "qwT", nc.gpsimd)
        wwT = loadw(wwT_d, "wwT", nc.gpsimd)
        _c = X8_TILE_CHUNKS[0]
        for i, nch in list(enumerate(X8_TILE_CHUNKS))[1:]:
            nc.sync.dma_start(
                out=x8_t[i], in_=x8_d[:, _c * 1024:(_c + nch) * 1024])
            _c += nch

        # dummy first-use ops: the ACT table load (~1.3us) and the DVE
        # warm-up happen here, during the input-DMA wait, instead of on the
        # conv phase's critical path
        dum = const.tile([128, 8], F32, tag="dum")
        nc.scalar.activation(out=dum[:, 0:4], in_=dum[:, 4:8],
                             func=AF.Exp, scale=0.0)
        nc.vector.tensor_copy(dum[:, 4:8], dum[:, 0:4])

        # per-chunk K / W~T / Q tiles (fp8) for fine-grained dependencies
        k_ch = [big.tile([128, N_KT, 512], FP8, tag=f"k{ch}", name=f"k_{ch}")
                for ch in range(N_CH)]
        wT_ch = [big.tile([128, 4, CV], FP8, tag=f"wT{ch}", name=f"wT_{ch}")
                 for ch in range(N_CH)]
        q_ch = [big.tile([128, N_KT, 512], FP8, tag=f"q{ib}", name=f"q_{ib}")
                for ib in range(N_IB)]
        # the 64s denominator column (the Pool engine owns SBUF memsets)
        for ch in range(N_CH):
            nc.gpsimd.memset(wT_ch[ch][:, :, C:C + 1], WS)

        with tc.tile_pool(name="P1", bufs=4, space="PSUM") as P1, \
             tc.tile_pool(name="ppool", bufs=10) as ppool, \
             tc.tile_pool(name="opool", bufs=3) as opool:

            hh_ps_of = {}

            def emit_qconv(ib, pool=None):
                for mt in range(N_KT):
                    ps = (pool or P1).tile([128, 512], F32, tag="w",
                                           name=f"psq{ib}_{mt}")
                    nc.tensor.matmul(
                        ps,
                        lhsT=qwT[:, :, mt * 128:(mt + 1) * 128],
                        rhs=xch(ib)[:, :, :],
                        start=True, stop=True, perf_mode=DR,
                    )
                    nc.vector.tensor_scalar_add(
                        q_ch[ib][:, mt, :], ps, qb[:, mt:mt + 1])

            def emit_mm1_exp(ib, pair, on_dve=False):
                """Scores + exp for one key-tile PAIR of query block ib.
                Each key-tile half gets its OWN one-bank psum tile and its
                own exp instruction: the slot frees after a ~470ns half-exp
                instead of a 1.1-1.5us whole-pair exp, which is what keeps
                mm1(p+2) from stalling on the 4-slot score ring."""
                ch, pp = divmod(pair, 2)
                pt = ppool.tile([128, 2, 512], FP8, tag="pT",
                                name=f"pt{ib}_{pair}")
                for h in range(2):
                    jj = pp * 2 + h
                    att = P1.tile([128, 512], F32, tag="w",
                                  name=f"att{ib}_{pair}_{h}")
                    nc.tensor.matmul(
                        att,
                        lhsT=k_ch[ch][:, :, jj * 128:(jj + 1) * 128],
                        rhs=q_ch[ib][:, :, :],
                        start=True, stop=True, perf_mode=DR,
                    )
                    if on_dve:
                        # fp8-bit Schraudolph on the DVE
                        nc.vector.tensor_scalar(
                            pt[:, h, :].bitcast(U8), att, SCH_A8, SCH_B8,
                            op0=mybir.AluOpType.mult,
                            op1=mybir.AluOpType.add,
                        )
                    else:
                        nc.scalar.activation(out=pt[:, h, :], in_=att,
                                             func=AF.Exp, scale=ESCALE)
                return pt

            def emit_mm2(ib, pair, pt):
                ch, pp = divmod(pair, 2)
                for isl in range(4):
                    nc.tensor.matmul(
                        hh_ps_of[ib][isl],
                        lhsT=pt[:, :, isl * 128:(isl + 1) * 128],
                        rhs=wT_ch[ch][:, pp * 2:pp * 2 + 2, :],
                        start=(pair == 0),
                        stop=(pair == N_PAIR - 1),
                        perf_mode=DR,
                    )

            def emit_tail(ib):
                """Drain block ib's hh psum (unnormalized, incl 64s col) to
                bf16 SBUF and store it with one 2KB-line DMA.  Casts split
                DVE/ACT so neither engine stalls the next block's mm2."""
                ob = opool.tile([128, 4, CV], BF16, tag="ob", name=f"ob{ib}")
                for isl in range(4):
                    if isl < 2:
                        nc.vector.tensor_copy(ob[:, isl, :],
                                              hh_ps_of[ib][isl][:, :])
                    else:
                        nc.scalar.activation(out=ob[:, isl, :],
                                             in_=hh_ps_of[ib][isl][:, :],
                                             func=AF.Copy)
                nc.sync.dma_start(
                    out=out_d[:, ib * 4 * CV:(ib + 1) * 4 * CV], in_=ob)

            # software pipeline: mm2 trails mm1+exp by MM2_DEPTH pairs so
            # the PE never waits on an exp (ACT or the slower DVE path) or
            # on the previous block's tail casts freeing the hh banks.
            inflight = []

            def emit_pair_piped(ib, pair, on_dve=False):
                pt = emit_mm1_exp(ib, pair, on_dve)
                inflight.append((ib, pair, pt))
                if len(inflight) > MM2_DEPTH:
                    emit_mm2(*inflight.pop(0))

            def flush_pairs():
                while inflight:
                    emit_mm2(*inflight.pop(0))

            def emit_kconv(ch, pool=None):
                for mt in range(N_KT):
                    ps = (pool or P1).tile([128, 512], F32, tag="w",
                                           name=f"psk{ch}_{mt}")
                    nc.tensor.matmul(
                        ps,
                        lhsT=kwT[:, :, mt * 128:(mt + 1) * 128],
                        rhs=xch(ch)[:, :, :],
                        start=True, stop=True, perf_mode=DR,
                    )
                    nc.vector.tensor_copy(k_ch[ch][:, mt, :], ps)

            def emit_wconv(ch, pool=None):
                for half in range(2):
                    ps = (pool or P1).tile([128, 2, C], F32, tag="w",
                                           name=f"psw{ch}_{half}")
                    for j in range(2):
                        jj = half * 2 + j
                        nc.tensor.matmul(
                            ps[:, j, :],
                            lhsT=xch(ch)[:, :, jj * 128:(jj + 1) * 128],
                            rhs=wwT[:, :, :],
                            start=True, stop=True, perf_mode=DR,
                        )
                    nc.vector.tensor_copy(
                        wT_ch[ch][:, half * 2:half * 2 + 2, 0:C], ps)

            # ---- conv boot (chunks 0-2) ----
            # The hh accumulator banks are idle until the first mm2 (pair 0
            # pops at emission of pair MM2_DEPTH, i.e. chunk 3).  Borrow
            # them as a second psum ring: conv psums alternate P1/BOOT over
            # FOUR slots, so each slot's DVE consumer (copy / q-bias add)
            # drains long before the slot is reused -- no PE ping-pong, and
            # the HAM un-throttles early.  Exps here stay whole-on-ACT (the
            # DVE is saturated with copies until the convs end).
            with tc.tile_pool(name="boot", bufs=2, space="PSUM") as BOOT:
                # HAM warm-up: the PE would sit idle from the prologue gate
                # (~7us) until the first input lands (~9.5us), guaranteeing
                # a cold (1.2 GHz) conv phase.  Garbage matmuls during the
                # DMA wait trip the activity monitor so the real convs run
                # at 2.4 GHz from the start.  Inputs are uninitialized SBUF
                # (possibly NaN) -- the psum is never read.
                dps = BOOT.tile([128, 512], F32, tag="w", name="dps")
                for r in range(16):
                    nc.tensor.matmul(
                        dps,
                        lhsT=dumf[:, :, 0:128],
                        rhs=dumf[:, :, :],
                        start=True, stop=True, perf_mode=DR,
                    )
                emit_kconv(0, BOOT)
                emit_qconv(0, P1)
                emit_wconv(0, BOOT)
                emit_kconv(1, P1)
                emit_wconv(1, BOOT)
                for pair in (0, 1):
                    emit_pair_piped(0, pair)
                emit_kconv(2, P1)
                emit_wconv(2, BOOT)
                for pair in (2, 3):
                    emit_pair_piped(0, pair)

            with tc.tile_pool(name="hps", bufs=4, space="PSUM") as hps:

                def hh_alloc(ib):
                    hh_ps_of[ib] = [hps.tile([128, CV], F32, tag="hh",
                                             name=f"hh_ps{ib}_{isl}")
                                    for isl in range(4)]

                hh_alloc(0)
                for ch in range(3, N_CH):
                    emit_kconv(ch)
                    if ch == 6:
                        emit_qconv(1)
                    emit_wconv(ch)
                    for pair in range(2 * (ch - 1), 2 * ch):
                        emit_pair_piped(0, pair)
                for pair in range(2 * (N_CH - 1), N_PAIR):
                    emit_pair_piped(0, pair)

                # ---- remaining query blocks ----
                pending = 0
                for ib in range(1, N_IB):
                    hh_alloc(ib)
                    for pair in range(N_PAIR):
                        emit_pair_piped(ib, pair, on_dve=(pair in DVE_PAIRS))
                        if pending is not None and pair == MM2_DEPTH - 1:
                            # mm2(pending, 15) was just emitted; the casts
                            # drain its psum banks before mm2(ib, 0)
                            # overwrites them at pair==DEPTH.
                            emit_tail(pending)
                            pending = None
                        if pair == 11 and ib + 1 < N_IB:
                            emit_qconv(ib + 1)
                    pending = ib
                flush_pairs()
                emit_tail(pending)

    nc.compile()
    return nc


_NC_CACHE = None


def _get_nc():
    global _NC_CACHE
    if _NC_CACHE is None:
        _NC_CACHE = build_nc()
    return _NC_CACHE


def make_in_maps(x, temb, q_w, q_b, k_w, k_b, v_w, v_b, p_w, p_b, t_w, t_b):
    xf = np.asarray(x, np.float32).reshape(B, C, HW)
    temb = np.asarray(temb, np.float32)
    fp8 = ml_dtypes.float8_e4m3
    # host-side algebraic folds
    t = temb @ np.asarray(t_w, np.float32).T + np.asarray(t_b, np.float32)
    pw = np.asarray(p_w, np.float32)
    Ww = pw @ np.asarray(v_w, np.float32)

    def wpack(wT):
        # [C, C] -> [128, 2, 256] partition-major, flattened [128, 512]
        return np.ascontiguousarray(
            wT.reshape(N_KT, 128, C).transpose(1, 0, 2).reshape(128, 512)
        ).astype(fp8)

    common = {
        "kwT": wpack(np.asarray(k_w, np.float32).T * WS),
        "qwT": wpack(np.asarray(q_w, np.float32).T * WS),
        "wwT": wpack(Ww.T * WS),
    }
    in_maps = []
    for core in range(N_CORES):
        b, h = divmod(core, 2)
        m = dict(common)
        # rotate so this core's query half occupies columns 0..NQ-1;
        # key order is irrelevant (softmax sums over keys).
        xr = xf[b] if h == 0 else np.concatenate(
            [xf[b][:, NQ:], xf[b][:, :NQ]], axis=1)
        # partition-major: [p][chunk][ctile][col] so every DMA line is a
        # 2KB contiguous span per partition
        m["x8"] = np.ascontiguousarray(
            xr.reshape(N_KT, 128, N_CH, 512).transpose(1, 2, 0, 3)
        ).reshape(128, N_CH * 1024).astype(fp8)
        m["qb"] = np.ascontiguousarray(
            (WS * (np.asarray(q_b, np.float32) + t[b])).reshape(N_KT, 128).T)
        in_maps.append(m)
    return in_maps


def run(in_maps, trace=False):
    nc = _get_nc()
    return run_bass_kernel_spmd(nc, in_maps, core_ids=list(range(N_CORES)),
                                trace=trace)


def kernel(**inputs):
    in_maps = make_in_maps(**inputs)
    res = run(in_maps)
    x = np.asarray(inputs["x"], np.float32).reshape(B, C, HW)
    pw = np.asarray(inputs["p_w"], np.float32)
    rbias = (np.asarray(inputs["p_b"], np.float32)
             + pw @ np.asarray(inputs["v_b"], np.float32))
    out = x + rbias[None, :, None]
    for core in range(N_CORES):
        b, h = divmod(core, 2)
        a = res.results[core]["out"].astype(np.float32)
        a = a.reshape(128, N_IB, 4, CV)
        hh = a[..., :C] / a[..., C:]          # normalize by the 64s column
        # query within this core's half = ib*512 + isl*128 + p
        hq = hh.transpose(1, 2, 0, 3).reshape(NQ, C).T   # [C, NQ]
        out[b, :, h * NQ:(h + 1) * NQ] += hq
    return out.reshape(B, C, H, W)
